# revision 6
# baseline (speedup 1.0000x reference)
"""LocallyConnected2d (3x3, pad 1) Trainium2 kernel.

Problem: out[b,o,h,w] = sum_{c,k} x_pad[b,c,h+k//3,w+k%3] * W[o,c,h,w,k]
  x: [16, 64, 56, 56] f32, W: [1, 64, 64, 56, 56, 9] f32 -> out [16, 64, 56, 56] f32

Strategy (8 cores, H sharded, 7 rows/core):
  The weight (462 MB) is used exactly once per element -> memory bound.
  Host pre-permutes the weight into per-core contiguous SBUF slab layout so the
  device streams it with full-width [128, N] contiguous DMAs.

  Per output location (h, w): out_loc[o, b] = sum_{c,k} W[ck, o] * xpatch[ck, b]
  done as 5 fp32 matmuls accumulating into one PSUM tile [64(o), 16(b)]:
    - 4 matmuls with K=128: two 3x3-taps stacked on the partition dim.
      The upper 64 partitions hold shifted copies of x so that a single AP
      (common free offset across partitions) reads tap k on the lower half and
      tap k' on the upper half: tap pairs {0,3},{1,4},{2,5} need offset delta
      58 (one padded row), pair {6,7} needs delta 1.
    - 1 matmul with K=64 for tap 8, alternating partition halves by w parity so
      each half carries exactly 9 taps per w-pair (keeps the weight slab dense).
  The weight is the *stationary* operand: fp32 moving operands stream at 4
  cycles/row, while LDWEIGHTS loads 1 column/cycle, so the big operand must
  ride the weight-load path (x streams as the 16-wide moving operand).
"""

import numpy as np

B, C, O, H, W = 16, 64, 64, 56, 56
NCORES = 8
HPC = H // NCORES          # 7 output rows per core
XROWS = HPC + 2            # 9 padded-x rows per core
XW = W + 2                 # 58
BLK = XROWS * XW           # 522 floats per (b, c) x block
XCOPY = B * BLK            # 8352 floats per x copy per partition
SPR = 2                    # weight slabs per output row
NSLAB = HPC * SPR          # 14 weight slabs per core
WSLAB = W // SPR           # 28 w positions per slab
WPS = WSLAB // 2           # 14 w-pairs per slab -> 14*9*64 floats/partition

# slot mapping within a w-pair (9 slots of 64 floats per partition):
#   even w: slots 0-3 = chunks 0-3, slot 4 = tap8 (lower half: even w, upper: odd w)
#   odd  w: slots 5-8 = chunks 0-3
K_LO = [0, 1, 2, 6]        # lower-half tap per chunk 0-3
K_HI = [3, 4, 5, 7]        # upper-half tap per chunk 0-3

_CACHE = {}


def _host_prep(x, weight):
    """Build per-core device input arrays (layout transforms, host-side only)."""
    x = np.ascontiguousarray(x, dtype=np.float32)
    w0 = weight.reshape(O, C, H, W, 9).astype(np.float32, copy=False)

    xpad = np.zeros((B, C, H + 2, W + 2), np.float32)
    xpad[:, :, 1:-1, 1:-1] = x

    xs_list, ws_list = [], []
    for core in range(NCORES):
        h0 = core * HPC
        # x copies: [128, 2*XCOPY]; lower 64 partitions (c) = [plain, plain],
        # upper = [shift-by-1, shift-by-58]
        xc = xpad[:, :, h0:h0 + XROWS, :]                     # [B, C, 9, 58]
        plain = np.ascontiguousarray(xc.transpose(1, 0, 2, 3)).reshape(C, XCOPY)
        sh1 = np.zeros_like(plain)
        sh1[:, :-1] = plain[:, 1:]
        sh58 = np.zeros_like(plain)
        sh58[:, :-58] = plain[:, 58:]
        xdev = np.empty((128, 2 * XCOPY), np.float32)
        xdev[:64, :XCOPY] = plain
        xdev[:64, XCOPY:] = plain
        xdev[64:, :XCOPY] = sh1
        xdev[64:, XCOPY:] = sh58
        xs_list.append(xdev.reshape(128, 2 * B, BLK))

        # weight slabs: S[h, p=(s,c), wp, slot, o]
        wc = w0[:, :, h0:h0 + HPC, :, :]                       # [O, C, 7, 56, 9]
        wt = wc.transpose(2, 1, 3, 4, 0)                       # [7, C, 56, 9, O]
        we = wt[:, :, 0::2]                                    # [7, C, 28, 9, O] even w
        wo = wt[:, :, 1::2]
        S = np.empty((HPC, 128, W // 2, 9, O), np.float32)
        S[:, :64, :, 0:4] = we[:, :, :, K_LO, :]
        S[:, :64, :, 4] = we[:, :, :, 8, :]
        S[:, :64, :, 5:9] = wo[:, :, :, K_LO, :]
        S[:, 64:, :, 0:4] = we[:, :, :, K_HI, :]
        S[:, 64:, :, 4] = wo[:, :, :, 8, :]
        S[:, 64:, :, 5:9] = wo[:, :, :, K_HI, :]
        # split each row into SPR slabs of WPS w-pairs
        Sr = S.reshape(HPC, 128, SPR, WPS, 9, O).transpose(0, 2, 1, 3, 4, 5)
        ws_list.append(np.ascontiguousarray(Sr).reshape(NSLAB, 128, WPS * 9, O))
    return xs_list, ws_list


def _build_program(repeat=1):
    import concourse.mybir as mybir
    import concourse.tile as tile
    from concourse import bacc

    f32 = mybir.dt.float32
    nc = bacc.Bacc("TRN2", target_bir_lowering=False, debug=False,
                   num_devices=NCORES)
    xs = nc.dram_tensor("xs", [128, 2 * B, BLK], f32, kind="ExternalInput")
    ws = nc.dram_tensor("ws", [NSLAB, 128, WPS * 9, O], f32, kind="ExternalInput")
    out = nc.dram_tensor("out", [HPC, O, W, B], f32, kind="ExternalOutput")

    with tile.TileContext(nc) as tc:
        with tc.tile_pool(name="xp", bufs=1) as xpool, \
             tc.tile_pool(name="wp", bufs=3) as wpool, \
             tc.tile_pool(name="op", bufs=2) as opool, \
             tc.tile_pool(name="pp", bufs=8, space="PSUM") as ppool:

            def body(_iv=None):
                xt = xpool.tile([128, 2 * B, BLK], f32, name="xt")
                nc.sync.dma_start(xt[:], xs[:])
                for h in range(HPC):
                    ot = opool.tile([O, W, B], f32, name="ot")
                    for sub in range(SPR):
                        slab = h * SPR + sub
                        wt = wpool.tile([128, WPS * 9, O], f32, name="wt")
                        nc.sync.dma_start(wt[:], ws[slab])
                        for wl in range(WSLAB):
                            w = sub * WSLAB + wl
                            wp, par = wl // 2, w % 2
                            base = 5 * par
                            ps = ppool.tile([O, B], f32, name="ps")
                            # chunks 0-2: taps {t, t+3}, K=128, region 1 (delta 58)
                            for t in range(3):
                                q = wp * 9 + base + t
                                F = h * XW + w + t
                                nc.tensor.matmul(
                                    ps[:, :], wt[:, q:q + 1, :],
                                    xt[:, B:2 * B, F:F + 1],
                                    start=(t == 0), stop=False)
                            # chunk 3: taps {6,7}, K=128, region 0 (delta 1)
                            q = wp * 9 + base + 3
                            F = (h + 2) * XW + w
                            nc.tensor.matmul(
                                ps[:, :], wt[:, q:q + 1, :], xt[:, 0:B, F:F + 1],
                                start=False, stop=False)
                            # chunk 4: tap 8, K=64, half picked by w parity
                            q = wp * 9 + 4
                            F = (h + 2) * XW + w + 2
                            if par == 0:
                                nc.tensor.matmul(
                                    ps[:, :], wt[0:64, q:q + 1, :],
                                    xt[0:64, 0:B, F:F + 1],
                                    start=False, stop=True)
                            else:
                                nc.tensor.matmul(
                                    ps[:, :], wt[64:128, q:q + 1, :],
                                    xt[64:128, 0:B, F - 1:F],
                                    start=False, stop=True)
                            nc.vector.tensor_copy(ot[:, w:w + 1, :], ps[:, :])
                    nc.sync.dma_start(out[h], ot[:])

            if repeat > 1:
                with tc.For_i(0, repeat, 1):
                    body()
            else:
                body()
    nc.compile()
    return nc


def _get_program(repeat=1):
    key = ("nc", repeat)
    if key not in _CACHE:
        _CACHE[key] = _build_program(repeat)
    return _CACHE[key]


def run(x, weight, trace=False, repeat=1):
    from concourse.bass_utils import run_bass_kernel_spmd

    nc = _get_program(repeat)
    xs_list, ws_list = _host_prep(np.asarray(x), np.asarray(weight))
    in_maps = [{"xs": xs_list[i], "ws": ws_list[i]} for i in range(NCORES)]
    res = run_bass_kernel_spmd(nc, in_maps, core_ids=list(range(NCORES)),
                               trace=trace)
    parts = []
    for i in range(NCORES):
        oc = np.asarray(res.results[i]["out"])       # [HPC, O, W, B]
        parts.append(oc.transpose(3, 1, 0, 2))       # [B, O, HPC, W]
    full = np.concatenate(parts, axis=2)             # [B, O, H, W]
    return np.ascontiguousarray(full), res


def kernel(x, weight):
    out, _ = run(x, weight, trace=False)
    return out
